# revision 1
# baseline (speedup 1.0000x reference)
"""MinusAttention kernel for Trainium2 (8 NeuronCores, Bass/Tile).

Math: score[i,j] = (w.q_i - w.k_j + b) / sqrt(E) with causal mask.
Within a softmax row i, the w.q_i and b terms are constant across j and
cancel, so

    weights[i,j] = g_j / sum_{j'<=i} g_j',   g_j = exp(-w.k_j / sqrt(E))
    out[i,:]     = (sum_{j<=i} g_j V[j,:]) / (sum_{j<=i} g_j)

i.e. a causal cumulative weighted average of V -- O(S*E) per (b,h)
instead of O(L*S*E) -- and the output does not depend on queries at all.

Device kernel per core (4 of the 32 (b,h) pairs), natural layout
[s%128 partitions, (s//128, e) free], per pair:

  - sk[p,k]   = reduce_add_e(ktw[p,k,e])      # DVE; ktw host-prescaled by -w/sqrt(E)
  - g         = exp(sk)                       # ACT  [128,16]
  - wg        = vg * g                        # DVE TT, g broadcast along free;
                                              # vg col 64 is ones -> wg col 64 = g
  - per chunk c (4 blocks): PSUM_c = TriUT @ wg_c   (within-block prefix sums)
  - cw32      = copy(PSUM rows 96:128)        # ACT (PSUM reads must be 32-aligned)
  - bsT[k]    = cw32 row 31 of each block     # tiny SBUF->SBUF DMA
  - rhs_m     = maskT * bsT_bcast             # GPSIMD; maskT[k',k]=1 iff k'<k
  - PSUM_c   += ones16 @ rhs_m_c              # adds carry_k = sum_{k'<k} bs_k'
  - cw        = copy(PSUM)                    # ACT -> SBUF
  - r         = 1/cw[:, :, 64]                # DVE [128,16]
  - out       = cw[:, :, 0:64] * r_bcast      # DVE TT

Pairs are processed in two groups of two with phase-major emission
(wavefront pipelining across engines, dense PE bursts); each pair's kt
streams on the SP HWDGE ring while vg streams on the ACT ring.
"""

import numpy as np

B, L, S, H, E = 4, 2048, 2048, 8, 64
NCORES = 8
PAIRS = (B * H) // NCORES  # (b,h) pairs per core
NBLK = S // 128  # 16
CHUNK = 4  # blocks per PSUM tile: 4*65 = 260 fp32 < 512 (one bank)
NCHUNK = NBLK // CHUNK  # 4
GROUP = 2  # pairs per phase-major group
SCALE = np.float32(1.0 / np.sqrt(np.float32(E)))

TRACE = False
LAST_RESULTS = None

_compiled = None


def _build():
    from concourse import bacc
    import concourse.mybir as mybir
    import concourse.tile as tile
    from concourse.masks import make_upper_triangular
    from concourse.tile_rust import add_dep_helper

    f32 = mybir.dt.float32
    nc = bacc.Bacc("TRN2", target_bir_lowering=False, debug=False)

    ktw = nc.dram_tensor("ktw", [PAIRS, 128, NBLK, E], f32, kind="ExternalInput")
    vg = nc.dram_tensor("vg", [PAIRS, 128, NBLK, E + 1], f32, kind="ExternalInput")
    out = nc.dram_tensor("out", [PAIRS, 128, NBLK, E], f32, kind="ExternalOutput")

    with tile.TileContext(nc) as tc:
        with (
            tc.tile_pool(name="const", bufs=1) as cpool,
            tc.tile_pool(name="ktp", bufs=2 * GROUP) as ktp,
            tc.tile_pool(name="vgp", bufs=2 * GROUP) as vgp,
            tc.tile_pool(name="gp", bufs=2 * GROUP) as gp,
            tc.tile_pool(name="wgp", bufs=2 * GROUP) as wgp,
            tc.tile_pool(name="bsp", bufs=2 * GROUP) as bsp,
            tc.tile_pool(name="rmp", bufs=2 * GROUP * NCHUNK) as rmp,
            tc.tile_pool(name="cwp", bufs=2 * GROUP) as cwp,
            tc.tile_pool(name="rp", bufs=2 * GROUP) as rp,
            tc.tile_pool(name="outp", bufs=2 * GROUP) as outp,
            tc.tile_pool(name="ps", bufs=8, space="PSUM") as psp,
        ):
            tri = cpool.tile([128, 128], f32)
            make_upper_triangular(nc, tri[:], val=1.0, diag=True)
            ones16 = cpool.tile([16, 128], f32)
            nc.gpsimd.memset(ones16[:], 1.0)
            # maskT[k', k, n] = 1 iff k' < k (strictly below target block)
            maskT = cpool.tile([16, NBLK, E + 1], f32)
            nc.gpsimd.memset(maskT[:], 1.0)
            nc.gpsimd.affine_select(
                out=maskT[:],
                in_=maskT[:],
                compare_op=mybir.AluOpType.is_gt,
                fill=0.0,
                base=0,
                # expr = -k' + k > 0  <=>  k' < k
                pattern=[[1, NBLK], [0, E + 1]],
                channel_multiplier=-1,
            )

            prev_wmul = None
            for grp in range(PAIRS // GROUP):
                pairs = list(range(grp * GROUP, (grp + 1) * GROUP))

                kts, vgts = {}, {}
                for p in pairs:
                    kt = ktp.tile([128, NBLK, E], f32, tag="kt")
                    vgt = vgp.tile([128, NBLK, E + 1], f32, tag="vg")
                    # kt on the SP HWDGE ring, vg on the ACT ring: both of a
                    # pair's inputs stream in parallel, earlier pairs first
                    nc.sync.dma_start(out=kt[:], in_=ktw[p])
                    nc.scalar.dma_start(out=vgt[:], in_=vg[p])
                    kts[p], vgts[p] = kt, vgt

                wgs = {}
                for p in pairs:
                    g = gp.tile([128, NBLK], f32, tag="g")
                    red = nc.vector.tensor_reduce(
                        g[:], kts[p][:], mybir.AxisListType.X, mybir.AluOpType.add
                    )
                    if prev_wmul is not None:
                        # order-only edge: a pair's reduce (gated on its kt
                        # arrival) must not be scheduled ahead of the
                        # previous pair's Wmul in the DVE stream, or the
                        # first matmuls stall on late kt DMAs
                        add_dep_helper(red.ins, prev_wmul.ins, sync=False,
                                       reason="reduce after prev pair wmul")
                    nc.scalar.activation(g[:], g[:], mybir.ActivationFunctionType.Exp)
                    wg = wgp.tile([128, NBLK, E + 1], f32, tag="wg")
                    gb = g[:].to_broadcast([128, NBLK, E + 1])
                    prev_wmul = nc.vector.tensor_tensor(
                        out=wg[:], in0=vgts[p][:], in1=gb, op=mybir.AluOpType.mult
                    )
                    wgs[p] = wg

                pss = {}
                for p in pairs:
                    for c in range(NCHUNK):
                        ps = psp.tile([128, CHUNK, E + 1], f32, tag="ps")
                        nc.tensor.matmul(
                            ps[:], lhsT=tri[:],
                            rhs=wgs[p][:, c * CHUNK : (c + 1) * CHUNK, :],
                            start=True, stop=False, skip_group_check=True,
                        )
                        pss[(p, c)] = ps

                bsTs = {}
                for p in pairs:
                    bsT = bsp.tile([NBLK, 1, E + 1], f32, tag="bs")
                    for c in range(NCHUNK):
                        # block sums live in row 127 of each block's prefix
                        # sums; PSUM reads need 32-aligned bases: copy rows
                        # 96:128 to SBUF, partition-scatter row 31 via DMA
                        c32 = cwp.tile([32, CHUNK, E + 1], f32, tag="cw32")
                        nc.scalar.copy(c32[:], pss[(p, c)][96:128, :, :])
                        nc.sync.dma_start(
                            out=bsT[c * CHUNK : (c + 1) * CHUNK, :, :],
                            in_=c32[31:32, :, :],
                        )
                    bsTs[p] = bsT

                rms = {}
                for p in pairs:
                    chunks = []
                    for c in range(NCHUNK):
                        rm = rmp.tile([16, CHUNK, E + 1], f32, tag="rm")
                        # chunk c's carries only involve block sums k' < 4c+4,
                        # i.e. rows already delivered by bs chunks 0..c
                        nc.gpsimd.tensor_tensor(
                            out=rm[:],
                            in0=maskT[:, c * CHUNK : (c + 1) * CHUNK, :],
                            in1=bsTs[p][:].broadcast_to([NBLK, CHUNK, E + 1]),
                            op=mybir.AluOpType.mult,
                        )
                        chunks.append(rm)
                    rms[p] = chunks

                for p in pairs:
                    for c in range(NCHUNK):
                        nc.tensor.matmul(
                            pss[(p, c)][:], lhsT=ones16[:],
                            rhs=rms[p][c][:],
                            start=False, stop=True, skip_group_check=True,
                        )

                cws = {}
                for p in pairs:
                    cw = cwp.tile([128, NBLK, E + 1], f32, tag="cw")
                    for c in range(NCHUNK):
                        # PSUM drain on DVE: runs in parallel with ACT's c32
                        # copies of the next group's block-sum extraction
                        nc.vector.tensor_copy(
                            cw[:, c * CHUNK : (c + 1) * CHUNK, :], pss[(p, c)][:]
                        )
                    cws[p] = cw

                for p in pairs:
                    r = rp.tile([128, NBLK], f32, tag="r")
                    nc.vector.reciprocal(
                        r[:], cws[p][:, :, E : E + 1].rearrange("p k o -> p (k o)")
                    )
                    ot = outp.tile([128, NBLK, E], f32, tag="out")
                    rb = r[:].to_broadcast([128, NBLK, E])
                    nc.vector.tensor_tensor(
                        out=ot[:], in0=cws[p][:, :, 0:E], in1=rb, op=mybir.AluOpType.mult
                    )
                    nc.sync.dma_start(out=out[p], in_=ot[:])

    nc.compile()
    return nc


def _get_compiled():
    global _compiled
    if _compiled is None:
        _compiled = _build()
    return _compiled


def prep_inputs(keys: np.ndarray, values: np.ndarray, w_score: np.ndarray):
    """Host-side reshard: returns in_maps (list of 8 dicts)."""
    keys = np.asarray(keys, dtype=np.float32)
    values = np.asarray(values, dtype=np.float32)
    w = np.asarray(w_score, dtype=np.float32)

    # [B,S,H,E] -> [B,H,S,E] -> [B*H, NBLK, 128, E] -> [B*H, 128, NBLK, E]
    kt = keys.transpose(0, 2, 1, 3).reshape(B * H, NBLK, 128, E)
    kt = (kt * (-SCALE * w)).transpose(0, 2, 1, 3)

    v = values.transpose(0, 2, 1, 3).reshape(B * H, NBLK, 128, E)
    vg = np.concatenate([v, np.ones((B * H, NBLK, 128, 1), np.float32)], axis=-1)
    vg = vg.transpose(0, 2, 1, 3)  # [B*H, 128, NBLK, E+1]

    in_maps = []
    for c in range(NCORES):
        sl = slice(PAIRS * c, PAIRS * (c + 1))
        in_maps.append(
            {
                "ktw": np.ascontiguousarray(kt[sl]),
                "vg": np.ascontiguousarray(vg[sl]),
            }
        )
    return in_maps


def assemble_output(results) -> np.ndarray:
    # results[c]["out"]: [PAIRS, 128, NBLK, E]; s = 128*k + partition
    arr = np.stack([np.asarray(r["out"]) for r in results])  # [8, PAIRS, 128, NBLK, E]
    arr = arr.reshape(B * H, 128, NBLK, E).transpose(0, 2, 1, 3)  # [B*H, NBLK, 128, E]
    arr = arr.reshape(B, H, L, E).transpose(0, 2, 1, 3)  # [B, L, H, E]
    return np.ascontiguousarray(arr)


def kernel(queries=None, keys=None, values=None, w_score=None, b_score=None, attn_mask=None, **_):
    global LAST_RESULTS
    from concourse.bass_utils import run_bass_kernel_spmd

    nc = _get_compiled()
    in_maps = prep_inputs(keys, values, w_score)
    res = run_bass_kernel_spmd(nc, in_maps, core_ids=list(range(NCORES)), trace=TRACE)
    LAST_RESULTS = res
    return assemble_output(res.results)



# revision 8
# speedup vs baseline: 1.0101x; 1.0101x over previous
"""MinusAttention kernel for Trainium2 (8 NeuronCores, Bass/Tile).

Math: score[i,j] = (w.q_i - w.k_j + b) / sqrt(E) with causal mask.
Within a softmax row i the w.q_i and b terms are constant across j and
cancel, so

    weights[i,j] = g_j / sum_{j'<=i} g_j',   g_j = exp(-w.k_j / sqrt(E))
    out[i,:]     = (sum_{j<=i} g_j V[j,:]) / (sum_{j<=i} g_j)

i.e. a causal cumulative weighted average of V -- O(S*E) per (b,h) --
and the output does not depend on queries at all.

Device kernel per core (4 of the 32 (b,h) pairs), all fp16 IO:

  s = 128*k + (127 - row): row-REVERSED within each 128-block, so each
  block's running total lands on PSUM partition 0 (DMA-readable).
  Prefix = lower-triangular matmul (within block) + per-block carries.

  Layouts: kt[row, k, e] (e innermost, for the E-reduction);
  v/wg/cw/out [row, e, k] (k innermost) so the g/r broadcasts ride on a
  middle dim and every elementwise multiply runs in DVE 2x mode;
  PSUM [row, k, e] so row 0 scatters with contiguous descriptors.

  Per pair: sk = reduce_e(kt) via two fp16 halving adds + reduce;
  g = exp(sk) (ACT); wg = v*g (DVE 2x); two 512-col fp16 prefix matmuls
  (tri, lower-incl) into 2 PSUM banks; block totals = PSUM row 0,
  DMA-scattered to [16,64]; rm = strict-lower mask * bs (DVE 2x);
  carry matmuls (ones16 @ rm) accumulate into the same banks; ACT
  drains PSUM -> fp16 cw in (e,k) layout; out = cw * (1/den) (DVE 2x).

  Denominator runs as a separate tiny pipeline over g alone (one PSUM
  bank for all 4 pairs): prefix matmul, row-0 scatter, masked carry,
  reciprocal -> r[128, pair, k].
"""

import numpy as np

B, L, S, H, E = 4, 2048, 2048, 8, 64
NCORES = 8
PAIRS = (B * H) // NCORES  # 4 (b,h) pairs per core
NBLK = S // 128  # 16 blocks of 128 positions
DUOS = PAIRS // 2  # pairs processed two at a time
SCALE = np.float32(1.0 / np.sqrt(np.float32(E)))

TRACE = False
LAST_RESULTS = None

_compiled = None


def _build():
    from concourse import bacc
    import concourse.mybir as mybir
    import concourse.tile as tile
    from concourse.masks import make_lower_triangular

    f16 = mybir.dt.float16
    f32 = mybir.dt.float32
    nc = bacc.Bacc("TRN2", target_bir_lowering=False, debug=False)

    # (duo, row, pair-in-duo, ...) fp16; 4KB contiguous per partition line
    ktin = nc.dram_tensor("ktin", [DUOS, 128, 2, NBLK, E], f16, kind="ExternalInput")
    vin = nc.dram_tensor("vin", [DUOS, 128, 2, E, NBLK], f16, kind="ExternalInput")
    outT = nc.dram_tensor("outT", [DUOS, 128, 2, E, NBLK], f16, kind="ExternalOutput")

    with tile.TileContext(nc) as tc:
        with (
            tc.tile_pool(name="const", bufs=1) as cpool,
            tc.tile_pool(name="ktp", bufs=2) as ktp,
            tc.tile_pool(name="vp", bufs=2) as vp,
            tc.tile_pool(name="s1p", bufs=2) as s1p,
            tc.tile_pool(name="s2p", bufs=2) as s2p,
            tc.tile_pool(name="skp", bufs=2) as skp,
            tc.tile_pool(name="wgp", bufs=2) as wgp,
            tc.tile_pool(name="bs1p", bufs=2) as bs1p,
            tc.tile_pool(name="bsp", bufs=2) as bsp,
            tc.tile_pool(name="rmp", bufs=2) as rmp,
            tc.tile_pool(name="dbs1p", bufs=2) as dbs1p,
            tc.tile_pool(name="dbsp", bufs=2) as dbsp,
            tc.tile_pool(name="drmp", bufs=2) as drmp,
            tc.tile_pool(name="cwp", bufs=2) as cwp,
            tc.tile_pool(name="otp", bufs=2) as otp,
            tc.tile_pool(name="psp", bufs=3, space="PSUM") as psp,
            tc.tile_pool(name="dpsp", bufs=1, space="PSUM") as dpsp,
        ):
            # --- constants ---
            triL = cpool.tile([128, 128], f16)
            make_lower_triangular(nc, triL[:], val=1.0, diag=True)
            ones16 = cpool.tile([16, 128], f16)
            nc.gpsimd.memset(ones16[:], 1.0)
            # mask3[k', k, e] = 1 iff k' < k (strictly earlier block)
            mask3 = cpool.tile([16, NBLK, E], f16)
            nc.gpsimd.memset(mask3[:], 1.0)
            nc.gpsimd.affine_select(
                out=mask3[:], in_=mask3[:],
                compare_op=mybir.AluOpType.is_gt, fill=0.0, base=0,
                pattern=[[1, NBLK], [0, E]], channel_multiplier=-1,
            )
            # mask3d[k', j, k] = 1 iff k' < k (den carry, per duo)
            mask3d = cpool.tile([16, 2, NBLK], f16)
            nc.gpsimd.memset(mask3d[:], 1.0)
            nc.gpsimd.affine_select(
                out=mask3d[:], in_=mask3d[:],
                compare_op=mybir.AluOpType.is_gt, fill=0.0, base=0,
                pattern=[[0, 2], [1, NBLK]], channel_multiplier=-1,
            )
            G = cpool.tile([128, PAIRS, NBLK], f16)
            r = cpool.tile([128, PAIRS, NBLK], f16)
            den = dpsp.tile([128, PAIRS, NBLK], f32)  # one PSUM bank

            # --- stream all inputs up front (SP ring), duo 0 first ---
            kts, vs = [], []
            for d in range(DUOS):
                kt = ktp.tile([128, 2, NBLK, E], f16, tag="kt")
                v = vp.tile([128, 2, E, NBLK], f16, tag="v")
                nc.sync.dma_start(out=kt[:], in_=ktin[d])
                nc.sync.dma_start(out=v[:], in_=vin[d])
                kts.append(kt)
                vs.append(v)

            pss = {}
            wgs, bss, bshs, rms, cws = [], [], [], [], []

            for d in range(DUOS):
                kt, v = kts[d], vs[d]
                # sk = sum_e kt  (two fp16 halving adds, then fp32 reduce)
                s1 = s1p.tile([128, 2, NBLK, 32], f16, tag="s1")
                nc.vector.tensor_tensor(
                    out=s1[:], in0=kt[:, :, :, 0:32], in1=kt[:, :, :, 32:64],
                    op=mybir.AluOpType.add,
                )
                s2 = s2p.tile([128, 2, NBLK, 16], f16, tag="s2")
                nc.vector.tensor_tensor(
                    out=s2[:], in0=s1[:, :, :, 0:16], in1=s1[:, :, :, 16:32],
                    op=mybir.AluOpType.add,
                )
                sk = skp.tile([128, 2, NBLK], f32, tag="sk")
                nc.vector.tensor_reduce(
                    sk[:], s2[:], mybir.AxisListType.X, mybir.AluOpType.add
                )
                # g = exp(sk) -> fp16 (ACT)
                nc.scalar.activation(
                    G[:, 2 * d : 2 * d + 2, :], sk[:],
                    mybir.ActivationFunctionType.Exp,
                )
                # den prefix for this duo's two pairs (shared bank)
                nc.tensor.matmul(
                    den[:, 2 * d : 2 * d + 2, :], lhsT=triL[:],
                    rhs=G[:, 2 * d : 2 * d + 2, :],
                    start=True, stop=False, skip_group_check=True,
                )
                # wg = v * g  (g broadcast on middle dim -> 2x)
                wg = wgp.tile([128, 2, E, NBLK], f16, tag="wg")
                gb = (
                    G[:, 2 * d : 2 * d + 2, :]
                    .rearrange("p j (o k) -> p j o k", o=1)
                    .broadcast_to([128, 2, E, NBLK])
                )
                nc.vector.tensor_tensor(
                    out=wg[:], in0=v[:], in1=gb, op=mybir.AluOpType.mult
                )
                wgs.append(wg)

                # within-block prefix matmuls: PSUM (k, e), 512 cols per bank
                for j in range(2):
                    ps = psp.tile([128, NBLK, E], f32, tag="ps")  # 2 banks
                    rhs = wg[:, j].rearrange("p e k -> p k e")
                    nc.tensor.matmul(
                        ps[:, 0:8, :], lhsT=triL[:], rhs=rhs[:, 0:8, :],
                        start=True, stop=False, skip_group_check=True,
                    )
                    nc.tensor.matmul(
                        ps[:, 8:16, :], lhsT=triL[:], rhs=rhs[:, 8:16, :],
                        start=True, stop=False, skip_group_check=True,
                    )
                    pss[(d, j)] = ps

                # block totals live on PSUM row 0 (reversed rows). DMA and
                # GPSIMD can't read PSUM: ACT copies row 0 -> SBUF fp16,
                # then an SBUF->SBUF scatter puts them on 16 partitions
                bs1 = bs1p.tile([1, 2, NBLK, E], f16, tag="bs1")
                for j in range(2):
                    nc.scalar.copy(bs1[:, j], pss[(d, j)][0:1, :, :])
                bs = bsp.tile([16, 2, E], f16, tag="bs")
                for j in range(2):
                    nc.sync.dma_start(out=bs[:, j, :], in_=bs1[:, j])
                bss.append(bs)
                # den totals: tiny ACT copy (written (k,j)-ordered) + scatter
                dbs1 = dbs1p.tile([1, NBLK, 2], f16, tag="dbs1")
                nc.scalar.copy(
                    dbs1[:].rearrange("p k j -> p j k"),
                    den[0:1, 2 * d : 2 * d + 2, :],
                )
                dbs = dbsp.tile([16, 2], f16, tag="dbs")
                nc.scalar.dma_start(out=dbs[:], in_=dbs1[:])

                # rm[k', j, k, e] = mask3[k',k,e] * bs[k',j,e]  (2x)
                rm = rmp.tile([16, 2, NBLK, E], f16, tag="rm")
                nc.vector.tensor_tensor(
                    out=rm[:],
                    in0=mask3[:].rearrange("p (o k) e -> p o k e", o=1).broadcast_to(
                        [16, 2, NBLK, E]
                    ),
                    in1=bs[:].rearrange("p j (o e) -> p j o e", o=1).broadcast_to(
                        [16, 2, NBLK, E]
                    ),
                    op=mybir.AluOpType.mult,
                )
                rms.append(rm)
                # den carry rm (tiny, 1x)
                drm = drmp.tile([16, 2, NBLK], f16, tag="drm")
                nc.vector.tensor_tensor(
                    out=drm[:],
                    in0=mask3d[:],
                    in1=dbs[:].rearrange("p (j o) -> p j o", o=1).broadcast_to(
                        [16, 2, NBLK]
                    ),
                    op=mybir.AluOpType.mult,
                )

                # carry matmuls accumulate into the same banks
                for j in range(2):
                    nc.tensor.matmul(
                        pss[(d, j)][:, 0:8, :], lhsT=ones16[:],
                        rhs=rms[d][:, j, 0:8, :],
                        start=False, stop=True, skip_group_check=True,
                    )
                    nc.tensor.matmul(
                        pss[(d, j)][:, 8:16, :], lhsT=ones16[:],
                        rhs=rms[d][:, j, 8:16, :],
                        start=False, stop=True, skip_group_check=True,
                    )
                nc.tensor.matmul(
                    den[:, 2 * d : 2 * d + 2, :], lhsT=ones16[:], rhs=drm[:],
                    start=False, stop=True, skip_group_check=True,
                )
                with nc.allow_low_precision("fp16 reciprocal feeds fp16 output"):
                    nc.vector.reciprocal(
                        r[:, 2 * d : 2 * d + 2, :], den[:, 2 * d : 2 * d + 2, :]
                    )

                # ACT drains PSUM -> fp16 cw in (e,k) layout
                cw = cwp.tile([128, 2, E, NBLK], f16, tag="cw")
                for j in range(2):
                    nc.scalar.copy(
                        cw[:, j].rearrange("p e k -> p k e"), pss[(d, j)][:]
                    )
                cws.append(cw)

                # out = cw * r  (r broadcast on middle dim -> 2x)
                ot = otp.tile([128, 2, E, NBLK], f16, tag="ot")
                rb = (
                    r[:, 2 * d : 2 * d + 2, :]
                    .rearrange("p j (o k) -> p j o k", o=1)
                    .broadcast_to([128, 2, E, NBLK])
                )
                nc.vector.tensor_tensor(
                    out=ot[:], in0=cw[:], in1=rb, op=mybir.AluOpType.mult
                )
                nc.sync.dma_start(out=outT[d], in_=ot[:])

    nc.compile()
    return nc


def _get_compiled():
    global _compiled
    if _compiled is None:
        _compiled = _build()
    return _compiled


def prep_inputs(keys: np.ndarray, values: np.ndarray, w_score: np.ndarray):
    """Host-side reshard: returns in_maps (list of 8 dicts)."""
    keys = np.asarray(keys, dtype=np.float32)
    values = np.asarray(values, dtype=np.float32)
    w = np.asarray(w_score, dtype=np.float32)

    # [B,S,H,E] -> [B*H, NBLK, 128, E], rows reversed within each block
    kt = keys.transpose(0, 2, 1, 3).reshape(B * H, NBLK, 128, E)[:, :, ::-1, :]
    kt = (kt * (-SCALE * w)).astype(np.float16)
    # -> [B*H, 128, NBLK, E]  (row, k, e)
    kt = kt.transpose(0, 2, 1, 3)

    v = values.transpose(0, 2, 1, 3).reshape(B * H, NBLK, 128, E)[:, :, ::-1, :]
    v = v.astype(np.float16).transpose(0, 2, 3, 1)  # [B*H, 128, E, NBLK]

    in_maps = []
    for c in range(NCORES):
        sl = slice(PAIRS * c, PAIRS * (c + 1))
        ktc = kt[sl]  # [4, 128, NBLK, E]
        vc = v[sl]  # [4, 128, E, NBLK]
        # [duo, row, pair-in-duo, ...]
        ktc = np.ascontiguousarray(
            ktc.reshape(DUOS, 2, 128, NBLK, E).transpose(0, 2, 1, 3, 4)
        )
        vc = np.ascontiguousarray(
            vc.reshape(DUOS, 2, 128, E, NBLK).transpose(0, 2, 1, 3, 4)
        )
        in_maps.append({"ktin": ktc, "vin": vc})
    return in_maps


def assemble_output(results) -> np.ndarray:
    # results[c]["outT"]: [DUOS, 128, 2, E, NBLK]; s = 128*k + (127-row)
    arr = np.stack([np.asarray(r["outT"]) for r in results])  # [8,D,128,2,E,K]
    arr = arr.transpose(0, 1, 3, 2, 4, 5).reshape(B * H, 128, E, NBLK)
    arr = arr.transpose(0, 3, 1, 2)[:, :, ::-1, :]  # [BH, k, row_rev, E]
    arr = arr.reshape(B, H, L, E).transpose(0, 2, 1, 3).astype(np.float32)
    return np.ascontiguousarray(arr)


def kernel(queries=None, keys=None, values=None, w_score=None, b_score=None, attn_mask=None, **_):
    global LAST_RESULTS
    from concourse.bass_utils import run_bass_kernel_spmd

    nc = _get_compiled()
    in_maps = prep_inputs(keys, values, w_score)
    res = run_bass_kernel_spmd(nc, in_maps, core_ids=list(range(NCORES)), trace=TRACE)
    LAST_RESULTS = res
    return assemble_output(res.results)
